# revision 13
# baseline (speedup 1.0000x reference)
"""Block-sparse flash attention (Phi-3-small pattern) on 8 Trainium2 cores.

Problem: S=2048 tokens, 32 query heads, 8 KV heads (GQA x4), D=128,
sparse_block_size=64, local_blocks=16, vert_stride=8, per-head vertical
offset (homo_head=False).

Sharding: tensor-parallel over heads. Core r owns contiguous heads
[4r, 4r+4), which all share GQA KV head r.

Per-head block mask (head h, c = (7-h) % 8):
  block (qb, kb) active iff qb >= kb and (qb-kb < 16 or kb % 8 == c)
Decomposition used here (verified exact vs reference on host):
  - LOCAL pass, k-tile kt covers kbs {2kt, 2kt+1}: q in [128kt, 128kt+1088)
      * causal triangle inside the diagonal 128x128 block
      * -inf on k-rows [0:64) for the last 64 q cols (qb-kb == 16 corner)
  - TAIL pass: the two vertical kbs {c, c+8} gathered on the host into one
    128-row k-tile; q in [1024, 2048) with a per-head rank-2 additive bias
    (rows 0:64 active for q >= 1024+64c, rows 64:128 for q >= 1536+64c).

All masks are applied as additive -1e5 biases ACCUMULATED INTO THE SCORES
PSUM BY PE MATMULS (identity x tribias for the triangle; rank-1/2 biases
for corner/tail), which keeps every instruction within the hardware's
sync-wait slot budget.

Kernel math (scores bounded, so softmax without max-subtraction is exact
to ~1e-6):  scoresT[k,q] on PE (contraction D=128 on partitions, so PV
needs no transposes), E^T = exp(SCALE*scoresT) on ACT (bf16, 1024-wide
chunks to amortize ACT instruction overhead), out^T accumulated in PSUM
over k-tiles, rowsum via ones-matmul, final PE transpose (bf16) +
per-partition 1/rowsum scale on DVE.

All per-head pattern differences are input DATA, so the single SPMD
program is identical on all 8 cores.
"""

import sys
from contextlib import ExitStack

import numpy as np

for _p in ("/opt/trn_rl_repo", "/root/.axon_site/_ro/trn_rl_repo"):
    if _p not in sys.path:
        sys.path.append(_p)

import ml_dtypes

import concourse.bass as bass
import concourse.bacc as bacc
import concourse.mybir as mybir
import concourse.tile as tile
from concourse.bass_utils import run_bass_kernel_spmd

S = 2048
D = 128
H = 32
HKV = 8
NCORES = 8
NH = H // NCORES          # heads per core = 4
SCALE = 0.08838834764831845
NKT = S // 128            # 16 k-tiles of 128 rows
SPAN = 1088               # local window cols per k-tile (17 blocks of 64)
HALF = 1024
WIN = 512                 # PSUM bank window (outT / rs)

BF16 = mybir.dt.bfloat16
F32 = mybir.dt.float32
NPBF16 = ml_dtypes.bfloat16


def _span_for(kt, half):
    """Local span of k-tile kt clipped to a q-half (always <= 1024 wide)."""
    lo = max(128 * kt, HALF * half)
    hi = min(128 * kt + SPAN, S, HALF * half + HALF)
    return (lo, hi) if lo < hi else None


def build_program(loop_n=1):
    nc = bacc.Bacc("TRN2", target_bir_lowering=False, debug=False)
    qT = nc.dram_tensor("qT", [NH, 128, S], BF16, kind="ExternalInput").ap()
    kT = nc.dram_tensor("kT", [128, S], BF16, kind="ExternalInput").ap()
    vR = nc.dram_tensor("vR", [128, S], BF16, kind="ExternalInput").ap()
    kvT = nc.dram_tensor("kvT", [NH, 128, 128], BF16, kind="ExternalInput").ap()
    vv = nc.dram_tensor("vv", [NH, 128, 128], BF16, kind="ExternalInput").ap()
    tb = nc.dram_tensor("tbias", [NH, 2, HALF], BF16, kind="ExternalInput").ap()
    trb = nc.dram_tensor("tribias", [128, 128], BF16, kind="ExternalInput").ap()
    idb = nc.dram_tensor("identb", [128, 128], BF16, kind="ExternalInput").ap()
    u2 = nc.dram_tensor("u2", [2, 128], BF16, kind="ExternalInput").ap()
    cb = nc.dram_tensor("cb", [2, 64], BF16, kind="ExternalInput").ap()
    out = nc.dram_tensor("out", [S, NH * 128], F32, kind="ExternalOutput").ap()

    Exp = mybir.ActivationFunctionType.Exp

    with tile.TileContext(nc) as tc, ExitStack() as ctx:
        const = ctx.enter_context(tc.tile_pool(name="const", bufs=1))
        perhead = ctx.enter_context(tc.tile_pool(name="perhead", bufs=2))
        eTp = ctx.enter_context(tc.tile_pool(name="eT", bufs=6))
        osb = ctx.enter_context(tc.tile_pool(name="osb", bufs=3))
        smal = ctx.enter_context(tc.tile_pool(name="small", bufs=2))
        scp = ctx.enter_context(tc.tile_pool(name="scores", bufs=2, space="PSUM"))
        otp = ctx.enter_context(tc.tile_pool(name="outT", bufs=3, space="PSUM"))
        rsp = ctx.enter_context(tc.tile_pool(name="rs", bufs=1, space="PSUM"))
        drp = ctx.enter_context(tc.tile_pool(name="dram", bufs=2, space="DRAM"))

        kT_sb = const.tile([128, S], BF16, tag="kT")
        nc.sync.dma_start(kT_sb[:], kT[:])
        v_sb = const.tile([128, S], BF16, tag="v")
        nc.sync.dma_start(v_sb[:], vR[:])
        trb_sb = const.tile([128, 128], BF16, tag="trb")
        nc.sync.dma_start(trb_sb[:], trb[:])
        idb_sb = const.tile([128, 128], BF16, tag="idb")
        nc.sync.dma_start(idb_sb[:], idb[:])
        u2_sb = const.tile([2, 128], BF16, tag="u2")
        nc.sync.dma_start(u2_sb[:], u2[:])
        ones_sb = const.tile([128, 1], BF16, tag="ones")
        nc.vector.memset(ones_sb[:], 1.0)
        onef_sb = const.tile([1, 1], F32, tag="onef")
        nc.vector.memset(onef_sb[:], 1.0)
        cb_sb = const.tile([2, 64], BF16, tag="cb")
        nc.sync.dma_start(cb_sb[:], cb[:])

        loop_cm = (tc.For_i(0, loop_n, 1,
                            hint_engines=(mybir.EngineType.PE,
                                          mybir.EngineType.Activation,
                                          mybir.EngineType.DVE,
                                          mybir.EngineType.SP))
                   if loop_n > 1 else None)
        if loop_cm is not None:
            loop_cm.__enter__()
        # Software pipeline: defer each chunk's PV/RS (and each half's
        # epilogue) by one stage so the in-order PE never sits waiting for
        # the exp of the chunk it just computed.
        pending = []

        def flush_one():
            if pending:
                pending.pop(0)()

        for h in range(NH):
            qT_sb = perhead.tile([128, S], BF16, tag="qT")
            nc.sync.dma_start(qT_sb[:], qT[h])
            kvT_sb = perhead.tile([128, 128], BF16, tag="kvT")
            nc.sync.dma_start(kvT_sb[:], kvT[h])
            vv_sb = perhead.tile([128, 128], BF16, tag="vv")
            nc.sync.dma_start(vv_sb[:], vv[h])
            tb_sb = perhead.tile([2, HALF], BF16, tag="tb")
            nc.sync.dma_start(tb_sb[:], tb[h])

            for half in (0, 1):
                half_lo = HALF * half
                half_hi = half_lo + HALF

                # ---- plan: one scores/eT chunk per (kt, half) + tail ----
                steps = []   # (kind, kt, a, b)
                for kt in range(NKT):
                    sp = _span_for(kt, half)
                    if sp is not None:
                        steps.append(("loc", kt, sp[0], sp[1]))
                if half == 1:
                    steps.append(("tail", -1, 1024, 2048))

                # PV/RS parts per step: split chunk at the outT 512-windows
                # and at the coverage boundary (fresh vs accumulating PSUM).
                def parts_of(kind, kt, a, b):
                    cov = half_lo
                    if kind == "tail":
                        cov = half_hi
                    elif kt > 0:
                        cov = min(max(min(1088 + 128 * (kt - 1), S), half_lo),
                                  half_hi)
                    cuts = {a, b, cov}
                    for wb in range(half_lo, half_hi + 1, WIN):
                        cuts.add(wb)
                    cuts = sorted(c for c in cuts if a <= c <= b)
                    return [(lo, hi) for lo, hi in zip(cuts, cuts[1:])
                            if lo < hi]

                n_into_w = [0, 0]
                all_parts = []
                for (kind, kt, a, b) in steps:
                    ps = parts_of(kind, kt, a, b)
                    all_parts.append(ps)
                    for (lo, hi) in ps:
                        n_into_w[(lo - half_lo) // WIN] += 1

                ow = [otp.tile([128, WIN], F32, tag="ow", name=f"ow{w}")
                      for w in range(2)]
                rs_c = rsp.tile([128, WIN], F32, tag="rs", name="rs_c")
                w_started = [False, False]
                w_seen = [0, 0]

                # ---- emit (stage A now, stage B deferred) ----
                for (kind, kt, a, b), ps in zip(steps, all_parts):
                    n = b - a
                    sc = scp.tile([128, HALF], F32, tag="sc")
                    if kind == "loc":
                        lhs_qk = kT_sb[:, 128 * kt:128 * kt + 128]
                        has_tri = a == 128 * kt and kt // 8 == half
                        has_cor = kt <= 7 and b == 128 * kt + 1088
                        has_tail = False
                    else:
                        lhs_qk = kvT_sb[:]
                        has_tri = has_cor = False
                        has_tail = True
                    cor_rel = (128 * kt + 1024) - a if has_cor else -1

                    # per-PSUM-bank writer lists (bank = 512-col half of sc)
                    # entries: (col_lo, col_hi, emit_fn) relative to a
                    def bank_of(lo):
                        return lo // WIN
                    writers = [[], []]
                    for s0 in range(0, n, WIN):
                        s1 = min(s0 + WIN, n)
                        writers[bank_of(s0)].append(
                            ("qk", s0, s1))
                    if has_tri:
                        writers[0].append(("tri", 0, 128))
                    if has_cor:
                        writers[bank_of(cor_rel)].append(
                            ("cor", cor_rel, cor_rel + 64))
                    if has_tail:
                        for s0 in range(0, n, WIN):
                            s1 = min(s0 + WIN, n)
                            writers[bank_of(s0)].append(("tb", s0, s1))
                    for bank in (0, 1):
                        wl = writers[bank]
                        for i, (wkind, s0, s1) in enumerate(wl):
                            st = wkind == "qk"
                            sp_f = i == len(wl) - 1
                            if wkind == "qk":
                                nc.tensor.matmul(sc[:, s0:s1], lhs_qk,
                                                 qT_sb[:, a + s0:a + s1],
                                                 start=True, stop=sp_f)
                            elif wkind == "tri":
                                nc.tensor.matmul(sc[:, s0:s1], idb_sb[:],
                                                 trb_sb[:], start=False,
                                                 stop=sp_f)
                            elif wkind == "cor":
                                nc.tensor.matmul(sc[:, s0:s1], u2_sb[:],
                                                 cb_sb[:], start=False,
                                                 stop=sp_f)
                            else:
                                nc.tensor.matmul(
                                    sc[:, s0:s1], u2_sb[:],
                                    tb_sb[:, a - HALF + s0:a - HALF + s1],
                                    start=False, stop=sp_f)

                    eT = eTp.tile([128, HALF], BF16, tag="eT")
                    nc.scalar.activation(eT[:, 0:n], sc[:, 0:n], Exp,
                                         scale=SCALE)
                    lhs_pv = (v_sb[:, 128 * kt:128 * kt + 128]
                              if kind == "loc" else vv_sb[:])

                    def stage_b(ps=ps, a=a, eT=eT, lhs_pv=lhs_pv, ow=ow,
                                rs_c=rs_c, w_started=w_started, w_seen=w_seen,
                                n_into_w=n_into_w, half_lo=half_lo):
                        for (lo, hi) in ps:
                            w = (lo - half_lo) // WIN
                            wl0 = half_lo + WIN * w
                            st = not w_started[w]
                            w_started[w] = True
                            w_seen[w] += 1
                            sp_f = w_seen[w] == n_into_w[w]
                            nc.tensor.matmul(ow[w][:, lo - wl0:hi - wl0],
                                             lhs_pv, eT[:, lo - a:hi - a],
                                             start=st, stop=sp_f)
                            nc.tensor.matmul(
                                rs_c[32 * w:32 * w + 1, lo - wl0:hi - wl0],
                                ones_sb[:, 0:1], eT[:, lo - a:hi - a],
                                start=st, stop=sp_f,
                                tile_position=(0, 32 * w) if w else None)

                    flush_one()
                    pending.append(stage_b)

                def epilogue(h=h, half_lo=half_lo, ow=ow, rs_c=rs_c):
                    for w in range(2):
                        rs_row = smal.tile([1, WIN], F32, tag="rsrow",
                                           name=f"rsrow{w}")
                        nc.vector.tensor_copy(rs_row[0:1, :],
                                              rs_c[32 * w:32 * w + 1, :])
                        rsT = scp.tile([128, 4], F32, tag="sc", name=f"rsT{w}")
                        for j in range(4):
                            nc.tensor.transpose(
                                rsT[:, j:j + 1],
                                rs_row[0:1, 128 * j:128 * j + 128], onef_sb[:])
                        rcp = smal.tile([128, 4], F32, tag="rcp",
                                        name=f"rcp{w}")
                        nc.vector.reciprocal(rcp[:], rsT[:])
                        ocp = osb.tile([128, WIN], BF16, tag="ocp")
                        nc.vector.tensor_copy(ocp[:], ow[w][:])
                        for j in range(4):
                            tp = scp.tile([128, 128], BF16, tag="sc",
                                          name=f"tp{w}{j}")
                            nc.tensor.transpose(
                                tp[:], ocp[:, 128 * j:128 * j + 128],
                                idb_sb[:])
                            os_t = osb.tile([128, 128], F32, tag="os")
                            nc.vector.tensor_scalar_mul(os_t[:], tp[:],
                                                        rcp[:, j:j + 1])
                            q0 = half_lo + WIN * w + 128 * j
                            nc.sync.dma_start(
                                out[q0:q0 + 128, 128 * h:128 * h + 128],
                                os_t[:])

                pending.append(epilogue)
        while pending:
            flush_one()
        if loop_cm is not None:
            loop_cm.__exit__(None, None, None)
    nc.compile()
    return nc


def make_core_inputs(query, key, value, core):
    """Host-side prep of one core's input map (bf16, pre-transposed/gathered)."""
    q3 = query.reshape(S, H, D)
    k3 = key.reshape(S, HKV, D)
    v3 = value.reshape(S, HKV, D)
    r = core
    K = k3[:, r, :]                     # [S, 128]
    V = v3[:, r, :]
    KT = np.ascontiguousarray(K.T)      # [128, S]
    vRe = np.ascontiguousarray(
        V.reshape(NKT, 128, D).transpose(1, 0, 2).reshape(128, S))

    NEG = np.float32(-100000.0)
    qT = np.empty((NH, 128, S), NPBF16)
    kvT = np.empty((NH, 128, 128), NPBF16)
    vv = np.empty((NH, 128, 128), NPBF16)
    tbias = np.zeros((NH, 2, HALF), NPBF16)
    for hl in range(NH):
        hg = NH * r + hl
        c = (7 - hg) % 8
        qT[hl] = q3[:, hg, :].T.astype(NPBF16)
        kvT[hl, :, 0:64] = KT[:, 64 * c:64 * c + 64].astype(NPBF16)
        kvT[hl, :, 64:128] = KT[:, 64 * (c + 8):64 * (c + 8) + 64].astype(NPBF16)
        vv[hl, 0:64, :] = V[64 * c:64 * c + 64, :].astype(NPBF16)
        vv[hl, 64:128, :] = V[64 * (c + 8):64 * (c + 8) + 64, :].astype(NPBF16)
        qq = np.arange(HALF)
        tbias[hl, 0, :] = np.where(qq < 64 * c, NEG, 0.0).astype(NPBF16)
        tbias[hl, 1, :] = np.where(qq < 512 + 64 * c, NEG, 0.0).astype(NPBF16)

    kk = np.arange(128)[:, None]
    qq = np.arange(128)[None, :]
    tribias = np.where(qq >= kk, 0.0, NEG).astype(NPBF16)
    u2 = np.zeros((2, 128), NPBF16)
    u2[0, 0:64] = 1.0
    u2[1, 64:128] = 1.0

    return {
        "qT": qT,
        "kT": KT.astype(NPBF16),
        "vR": vRe.astype(NPBF16),
        "kvT": kvT,
        "vv": vv,
        "tbias": tbias,
        "tribias": tribias,
        "identb": np.eye(128, dtype=NPBF16),
        "u2": u2,
        "cb": np.concatenate([np.full((1, 64), NEG, NPBF16),
                              np.zeros((1, 64), NPBF16)], axis=0),
    }


_PROGRAM = None


def _get_program():
    global _PROGRAM
    if _PROGRAM is None:
        _PROGRAM = build_program()
    return _PROGRAM


def run(query, key, value, trace=False):
    """Returns (output [S, H*D] f32, BassKernelResults)."""
    nc = _get_program()
    in_maps = [make_core_inputs(query, key, value, r) for r in range(NCORES)]
    br = run_bass_kernel_spmd(nc, in_maps, list(range(NCORES)), trace=trace)
    outp = np.hstack([br.results[r]["out"] for r in range(NCORES)])
    return outp, br


def kernel(query, key, value):
    outp, _ = run(np.asarray(query), np.asarray(key), np.asarray(value))
    return outp


# revision 14
# speedup vs baseline: 1.0889x; 1.0889x over previous
"""Block-sparse flash attention (Phi-3-small pattern) on 8 Trainium2 cores.

Problem: S=2048 tokens, 32 query heads, 8 KV heads (GQA x4), D=128,
sparse_block_size=64, local_blocks=16, vert_stride=8, per-head vertical
offset (homo_head=False).

Sharding: tensor-parallel over heads. Core r owns contiguous heads
[4r, 4r+4), which all share GQA KV head r.

Per-head block mask (head h, c = (7-h) % 8):
  block (qb, kb) active iff qb >= kb and (qb-kb < 16 or kb % 8 == c)
Decomposition used here (verified exact vs reference on host):
  - LOCAL pass, k-tile kt covers kbs {2kt, 2kt+1}: q in [128kt, 128kt+1088)
      * causal triangle inside the diagonal 128x128 block
      * -inf on k-rows [0:64) for the last 64 q cols (qb-kb == 16 corner)
  - TAIL pass: the two vertical kbs {c, c+8} gathered on the host into one
    128-row k-tile; q in [1024, 2048) with a per-head rank-2 additive bias
    (rows 0:64 active for q >= 1024+64c, rows 64:128 for q >= 1536+64c).

All masks are applied as additive -1e5 biases ACCUMULATED INTO THE SCORES
PSUM BY PE MATMULS (identity x tribias for the triangle; rank-1/2 biases
for corner/tail), which keeps every instruction within the hardware's
sync-wait slot budget.

Kernel math (scores bounded, so softmax without max-subtraction is exact
to ~1e-6):  scoresT[k,q] on PE (contraction D=128 on partitions, so PV
needs no transposes), E^T = exp(SCALE*scoresT) on ACT (bf16, 1024-wide
chunks to amortize ACT instruction overhead), out^T accumulated in PSUM
over k-tiles, rowsum via ones-matmul, final PE transpose (bf16) +
per-partition 1/rowsum scale on DVE.

All per-head pattern differences are input DATA, so the single SPMD
program is identical on all 8 cores.
"""

import sys
from contextlib import ExitStack

import numpy as np

for _p in ("/opt/trn_rl_repo", "/root/.axon_site/_ro/trn_rl_repo"):
    if _p not in sys.path:
        sys.path.append(_p)

import ml_dtypes

import concourse.bass as bass
import concourse.bacc as bacc
import concourse.mybir as mybir
import concourse.tile as tile
from concourse.bass_utils import run_bass_kernel_spmd

S = 2048
D = 128
H = 32
HKV = 8
NCORES = 8
NH = H // NCORES          # heads per core = 4
SCALE = 0.08838834764831845
NKT = S // 128            # 16 k-tiles of 128 rows
SPAN = 1088               # local window cols per k-tile (17 blocks of 64)
HALF = 1024
WIN = 512                 # PSUM bank window (outT / rs)

BF16 = mybir.dt.bfloat16
F32 = mybir.dt.float32
NPBF16 = ml_dtypes.bfloat16


def _chunks_for(kt, half):
    """512-aligned chunks of the local span of k-tile kt inside a q-half."""
    lo = max(128 * kt, HALF * half)
    hi = min(128 * kt + SPAN, S, HALF * half + HALF)
    res = []
    a = lo
    while a < hi:
        b = min((a // WIN + 1) * WIN, hi)
        res.append((a, b))
        a = b
    return res


def build_program(loop_n=1):
    nc = bacc.Bacc("TRN2", target_bir_lowering=False, debug=False)
    qT = nc.dram_tensor("qT", [NH, 128, S], BF16, kind="ExternalInput").ap()
    kT = nc.dram_tensor("kT", [128, S], BF16, kind="ExternalInput").ap()
    vR = nc.dram_tensor("vR", [128, S], BF16, kind="ExternalInput").ap()
    kvT = nc.dram_tensor("kvT", [NH, 128, 128], BF16, kind="ExternalInput").ap()
    vv = nc.dram_tensor("vv", [NH, 128, 128], BF16, kind="ExternalInput").ap()
    tb = nc.dram_tensor("tbias", [NH, 2, HALF], BF16, kind="ExternalInput").ap()
    trb = nc.dram_tensor("tribias", [128, 128], BF16, kind="ExternalInput").ap()
    idb = nc.dram_tensor("identb", [128, 128], BF16, kind="ExternalInput").ap()
    u2 = nc.dram_tensor("u2", [2, 128], BF16, kind="ExternalInput").ap()
    cb = nc.dram_tensor("cb", [2, 64], BF16, kind="ExternalInput").ap()
    out = nc.dram_tensor("out", [S, NH * 128], F32, kind="ExternalOutput").ap()

    Exp = mybir.ActivationFunctionType.Exp

    with tile.TileContext(nc) as tc, ExitStack() as ctx:
        const = ctx.enter_context(tc.tile_pool(name="const", bufs=1))
        perhead = ctx.enter_context(tc.tile_pool(name="perhead", bufs=2))
        eTp = ctx.enter_context(tc.tile_pool(name="eT", bufs=6))
        osb = ctx.enter_context(tc.tile_pool(name="osb", bufs=3))
        smal = ctx.enter_context(tc.tile_pool(name="small", bufs=2))
        scp = ctx.enter_context(tc.tile_pool(name="scores", bufs=4, space="PSUM"))
        otp = ctx.enter_context(tc.tile_pool(name="outT", bufs=3, space="PSUM"))
        rsp = ctx.enter_context(tc.tile_pool(name="rs", bufs=1, space="PSUM"))
        drp = ctx.enter_context(tc.tile_pool(name="dram", bufs=2, space="DRAM"))

        kT_sb = const.tile([128, S], BF16, tag="kT")
        nc.sync.dma_start(kT_sb[:], kT[:])
        v_sb = const.tile([128, S], BF16, tag="v")
        nc.sync.dma_start(v_sb[:], vR[:])
        trb_sb = const.tile([128, 128], BF16, tag="trb")
        nc.sync.dma_start(trb_sb[:], trb[:])
        idb_sb = const.tile([128, 128], BF16, tag="idb")
        nc.sync.dma_start(idb_sb[:], idb[:])
        u2_sb = const.tile([2, 128], BF16, tag="u2")
        nc.sync.dma_start(u2_sb[:], u2[:])
        ones_sb = const.tile([128, 1], BF16, tag="ones")
        nc.vector.memset(ones_sb[:], 1.0)
        onef_sb = const.tile([1, 1], F32, tag="onef")
        nc.vector.memset(onef_sb[:], 1.0)
        cb_sb = const.tile([2, 64], BF16, tag="cb")
        nc.sync.dma_start(cb_sb[:], cb[:])

        loop_cm = (tc.For_i(0, loop_n, 1,
                            hint_engines=(mybir.EngineType.PE,
                                          mybir.EngineType.Activation,
                                          mybir.EngineType.DVE,
                                          mybir.EngineType.SP))
                   if loop_n > 1 else None)
        if loop_cm is not None:
            loop_cm.__enter__()
        # Software pipeline: defer each chunk's PV/RS (and each half's
        # epilogue) by one stage so the in-order PE never sits waiting for
        # the exp of the chunk it just computed.
        pending = []
        LAG = 2

        def flush_one(force=False):
            if pending and (force or len(pending) > LAG):
                pending.pop(0)()

        for h in range(NH):
            qT_sb = perhead.tile([128, S], BF16, tag="qT")
            nc.sync.dma_start(qT_sb[:], qT[h])
            kvT_sb = perhead.tile([128, 128], BF16, tag="kvT")
            nc.sync.dma_start(kvT_sb[:], kvT[h])
            vv_sb = perhead.tile([128, 128], BF16, tag="vv")
            nc.sync.dma_start(vv_sb[:], vv[h])
            tb_sb = perhead.tile([2, HALF], BF16, tag="tb")
            nc.sync.dma_start(tb_sb[:], tb[h])

            for half in (0, 1):
                half_lo = HALF * half
                half_hi = half_lo + HALF

                # ---- plan: 512-wide scores/eT chunks + tail ----
                steps = []   # (kind, kt, a, b)
                for kt in range(NKT):
                    for (a, b) in _chunks_for(kt, half):
                        steps.append(("loc", kt, a, b))
                if half == 1:
                    steps.append(("tail", -1, 1024, 1536))
                    steps.append(("tail", -1, 1536, 2048))

                # PV/RS parts per step: split chunk at the outT 512-windows
                # and at the coverage boundary (fresh vs accumulating PSUM).
                def parts_of(kind, kt, a, b):
                    cov = half_lo
                    if kind == "tail":
                        cov = half_hi
                    elif kt > 0:
                        cov = min(max(min(1088 + 128 * (kt - 1), S), half_lo),
                                  half_hi)
                    cuts = {a, b, cov}
                    for wb in range(half_lo, half_hi + 1, WIN):
                        cuts.add(wb)
                    cuts = sorted(c for c in cuts if a <= c <= b)
                    return [(lo, hi) for lo, hi in zip(cuts, cuts[1:])
                            if lo < hi]

                n_into_w = [0, 0]
                all_parts = []
                for (kind, kt, a, b) in steps:
                    ps = parts_of(kind, kt, a, b)
                    all_parts.append(ps)
                    for (lo, hi) in ps:
                        n_into_w[(lo - half_lo) // WIN] += 1

                ow = [otp.tile([128, WIN], F32, tag="ow", name=f"ow{w}")
                      for w in range(2)]
                rs_c = rsp.tile([128, WIN], F32, tag="rs", name="rs_c")
                w_started = [False, False]
                w_seen = [0, 0]

                # ---- emit (stage A now, stage B deferred) ----
                for (kind, kt, a, b), ps in zip(steps, all_parts):
                    n = b - a
                    sc = scp.tile([128, WIN], F32, tag="sc")
                    if kind == "loc":
                        lhs_qk = kT_sb[:, 128 * kt:128 * kt + 128]
                        has_tri = a == 128 * kt and kt // 8 == half
                        has_cor = kt <= 7 and b == 128 * kt + 1088
                        has_tail = False
                    else:
                        lhs_qk = kvT_sb[:]
                        has_tri = has_cor = False
                        has_tail = True
                    writers = [("qk", 0, n)]
                    if has_tri:
                        writers.append(("tri", 0, 128))
                    if has_cor:
                        rel = (128 * kt + 1024) - a
                        writers.append(("cor", rel, rel + 64))
                    if has_tail:
                        writers.append(("tb", 0, n))
                    for i, (wkind, s0, s1) in enumerate(writers):
                        sp_f = i == len(writers) - 1
                        if wkind == "qk":
                            nc.tensor.matmul(sc[:, s0:s1], lhs_qk,
                                             qT_sb[:, a + s0:a + s1],
                                             start=True, stop=sp_f)
                        elif wkind == "tri":
                            nc.tensor.matmul(sc[:, s0:s1], idb_sb[:],
                                             trb_sb[:], start=False,
                                             stop=sp_f)
                        elif wkind == "cor":
                            nc.tensor.matmul(sc[:, s0:s1], u2_sb[:],
                                             cb_sb[:], start=False,
                                             stop=sp_f)
                        else:
                            nc.tensor.matmul(
                                sc[:, s0:s1], u2_sb[:],
                                tb_sb[:, a - HALF + s0:a - HALF + s1],
                                start=False, stop=sp_f)

                    eT = eTp.tile([128, WIN], BF16, tag="eT")
                    nc.scalar.activation(eT[:, 0:n], sc[:, 0:n], Exp,
                                         scale=SCALE)
                    lhs_pv = (v_sb[:, 128 * kt:128 * kt + 128]
                              if kind == "loc" else vv_sb[:])

                    def stage_b(ps=ps, a=a, eT=eT, lhs_pv=lhs_pv, ow=ow,
                                rs_c=rs_c, w_started=w_started, w_seen=w_seen,
                                n_into_w=n_into_w, half_lo=half_lo):
                        for (lo, hi) in ps:
                            w = (lo - half_lo) // WIN
                            wl0 = half_lo + WIN * w
                            st = not w_started[w]
                            w_started[w] = True
                            w_seen[w] += 1
                            sp_f = w_seen[w] == n_into_w[w]
                            nc.tensor.matmul(ow[w][:, lo - wl0:hi - wl0],
                                             lhs_pv, eT[:, lo - a:hi - a],
                                             start=st, stop=sp_f)
                            nc.tensor.matmul(
                                rs_c[32 * w:32 * w + 1, lo - wl0:hi - wl0],
                                ones_sb[:, 0:1], eT[:, lo - a:hi - a],
                                start=st, stop=sp_f,
                                tile_position=(0, 32 * w) if w else None)

                    flush_one()
                    pending.append(stage_b)

                def epilogue(h=h, half_lo=half_lo, ow=ow, rs_c=rs_c):
                    for w in range(2):
                        rs_row = smal.tile([1, WIN], F32, tag="rsrow",
                                           name=f"rsrow{w}")
                        nc.vector.tensor_copy(rs_row[0:1, :],
                                              rs_c[32 * w:32 * w + 1, :])
                        rsT = scp.tile([128, 4], F32, tag="sc", name=f"rsT{w}")
                        for j in range(4):
                            nc.tensor.transpose(
                                rsT[:, j:j + 1],
                                rs_row[0:1, 128 * j:128 * j + 128], onef_sb[:])
                        rcp = smal.tile([128, 4], F32, tag="rcp",
                                        name=f"rcp{w}")
                        nc.vector.reciprocal(rcp[:], rsT[:])
                        ocp = osb.tile([128, WIN], BF16, tag="ocp")
                        nc.vector.tensor_copy(ocp[:], ow[w][:])
                        for j in range(4):
                            tp = scp.tile([128, 128], BF16, tag="sc",
                                          name=f"tp{w}{j}")
                            nc.tensor.transpose(
                                tp[:], ocp[:, 128 * j:128 * j + 128],
                                idb_sb[:])
                            os_t = osb.tile([128, 128], F32, tag="os")
                            nc.vector.tensor_scalar_mul(os_t[:], tp[:],
                                                        rcp[:, j:j + 1])
                            q0 = half_lo + WIN * w + 128 * j
                            nc.sync.dma_start(
                                out[q0:q0 + 128, 128 * h:128 * h + 128],
                                os_t[:])

                pending.append(epilogue)
        while pending:
            flush_one(force=True)
        if loop_cm is not None:
            loop_cm.__exit__(None, None, None)
    nc.compile()
    return nc


def make_core_inputs(query, key, value, core):
    """Host-side prep of one core's input map (bf16, pre-transposed/gathered)."""
    q3 = query.reshape(S, H, D)
    k3 = key.reshape(S, HKV, D)
    v3 = value.reshape(S, HKV, D)
    r = core
    K = k3[:, r, :]                     # [S, 128]
    V = v3[:, r, :]
    KT = np.ascontiguousarray(K.T)      # [128, S]
    vRe = np.ascontiguousarray(
        V.reshape(NKT, 128, D).transpose(1, 0, 2).reshape(128, S))

    NEG = np.float32(-100000.0)
    qT = np.empty((NH, 128, S), NPBF16)
    kvT = np.empty((NH, 128, 128), NPBF16)
    vv = np.empty((NH, 128, 128), NPBF16)
    tbias = np.zeros((NH, 2, HALF), NPBF16)
    for hl in range(NH):
        hg = NH * r + hl
        c = (7 - hg) % 8
        qT[hl] = q3[:, hg, :].T.astype(NPBF16)
        kvT[hl, :, 0:64] = KT[:, 64 * c:64 * c + 64].astype(NPBF16)
        kvT[hl, :, 64:128] = KT[:, 64 * (c + 8):64 * (c + 8) + 64].astype(NPBF16)
        vv[hl, 0:64, :] = V[64 * c:64 * c + 64, :].astype(NPBF16)
        vv[hl, 64:128, :] = V[64 * (c + 8):64 * (c + 8) + 64, :].astype(NPBF16)
        qq = np.arange(HALF)
        tbias[hl, 0, :] = np.where(qq < 64 * c, NEG, 0.0).astype(NPBF16)
        tbias[hl, 1, :] = np.where(qq < 512 + 64 * c, NEG, 0.0).astype(NPBF16)

    kk = np.arange(128)[:, None]
    qq = np.arange(128)[None, :]
    tribias = np.where(qq >= kk, 0.0, NEG).astype(NPBF16)
    u2 = np.zeros((2, 128), NPBF16)
    u2[0, 0:64] = 1.0
    u2[1, 64:128] = 1.0

    return {
        "qT": qT,
        "kT": KT.astype(NPBF16),
        "vR": vRe.astype(NPBF16),
        "kvT": kvT,
        "vv": vv,
        "tbias": tbias,
        "tribias": tribias,
        "identb": np.eye(128, dtype=NPBF16),
        "u2": u2,
        "cb": np.concatenate([np.full((1, 64), NEG, NPBF16),
                              np.zeros((1, 64), NPBF16)], axis=0),
    }


_PROGRAM = None


def _get_program():
    global _PROGRAM
    if _PROGRAM is None:
        _PROGRAM = build_program()
    return _PROGRAM


def run(query, key, value, trace=False):
    """Returns (output [S, H*D] f32, BassKernelResults)."""
    nc = _get_program()
    in_maps = [make_core_inputs(query, key, value, r) for r in range(NCORES)]
    br = run_bass_kernel_spmd(nc, in_maps, list(range(NCORES)), trace=trace)
    outp = np.hstack([br.results[r]["out"] for r in range(NCORES)])
    return outp, br


def kernel(query, key, value):
    outp, _ = run(np.asarray(query), np.asarray(key), np.asarray(value))
    return outp


# revision 17
# speedup vs baseline: 1.3152x; 1.2078x over previous
"""Block-sparse flash attention (Phi-3-small pattern) on 8 Trainium2 cores.

Problem: S=2048 tokens, 32 query heads, 8 KV heads (GQA x4), D=128,
sparse_block_size=64, local_blocks=16, vert_stride=8, per-head vertical
offset (homo_head=False).

Sharding: tensor-parallel over heads. Core r owns contiguous heads
[4r, 4r+4), which all share GQA KV head r.

Per-head block mask (head h, c = (7-h) % 8):
  block (qb, kb) active iff qb >= kb and (qb-kb < 16 or kb % 8 == c)
Decomposition used here (verified exact vs reference on host):
  - LOCAL pass, k-tile kt covers kbs {2kt, 2kt+1}: q in [128kt, 128kt+1088)
      * causal triangle inside the diagonal 128x128 block
      * -inf on k-rows [0:64) for the last 64 q cols (qb-kb == 16 corner)
  - TAIL pass: the two vertical kbs {c, c+8} gathered on the host into one
    128-row k-tile; q in [1024, 2048) with a per-head rank-2 additive bias
    (rows 0:64 active for q >= 1024+64c, rows 64:128 for q >= 1536+64c).

All masks are applied as additive -1e5 biases ACCUMULATED INTO THE SCORES
PSUM BY PE MATMULS (identity x tribias for the triangle; rank-1/2 biases
for corner/tail), which keeps every instruction within the hardware's
sync-wait slot budget.

Kernel math (scores bounded, so softmax without max-subtraction is exact
to ~1e-6):  scoresT[k,q] on PE (contraction D=128 on partitions, so PV
needs no transposes), E^T = exp(SCALE*scoresT) on ACT (bf16, 1024-wide
chunks to amortize ACT instruction overhead), out^T accumulated in PSUM
over k-tiles, rowsum via ones-matmul, final PE transpose (bf16) +
per-partition 1/rowsum scale on DVE.

All per-head pattern differences are input DATA, so the single SPMD
program is identical on all 8 cores.
"""

import sys
from contextlib import ExitStack

import numpy as np

for _p in ("/opt/trn_rl_repo", "/root/.axon_site/_ro/trn_rl_repo"):
    if _p not in sys.path:
        sys.path.append(_p)

import ml_dtypes

import concourse.bass as bass
import concourse.bacc as bacc
import concourse.mybir as mybir
import concourse.tile as tile
from concourse.bass_utils import run_bass_kernel_spmd

S = 2048
D = 128
H = 32
HKV = 8
NCORES = 8
NH = H // NCORES          # heads per core = 4
SCALE = 0.08838834764831845
NKT = S // 128            # 16 k-tiles of 128 rows
SPAN = 1088               # local window cols per k-tile (17 blocks of 64)
HALF = 1024
WIN = 512                 # PSUM bank window (outT / rs)

BF16 = mybir.dt.bfloat16
F32 = mybir.dt.float32
NPBF16 = ml_dtypes.bfloat16


def _chunks_for(kt, half):
    """512-aligned chunks of the local span of k-tile kt inside a q-half."""
    lo = max(128 * kt, HALF * half)
    hi = min(128 * kt + SPAN, S, HALF * half + HALF)
    res = []
    a = lo
    while a < hi:
        b = min((a // WIN + 1) * WIN, hi)
        res.append((a, b))
        a = b
    return res


def build_program(loop_n=1, lag=2, scb=4, owb=3):
    nc = bacc.Bacc("TRN2", target_bir_lowering=False, debug=False)
    qT = nc.dram_tensor("qT", [NH, 128, S], BF16, kind="ExternalInput").ap()
    kT = nc.dram_tensor("kT", [128, S], BF16, kind="ExternalInput").ap()
    vR = nc.dram_tensor("vR", [128, S], BF16, kind="ExternalInput").ap()
    kvT = nc.dram_tensor("kvT", [NH, 128, 128], BF16, kind="ExternalInput").ap()
    vv = nc.dram_tensor("vv", [NH, 128, 128], BF16, kind="ExternalInput").ap()
    tb = nc.dram_tensor("tbias", [NH, 2, HALF], BF16, kind="ExternalInput").ap()
    trb = nc.dram_tensor("tribias", [128, 128], BF16, kind="ExternalInput").ap()
    idb = nc.dram_tensor("identb", [128, 128], BF16, kind="ExternalInput").ap()
    u2 = nc.dram_tensor("u2", [2, 128], BF16, kind="ExternalInput").ap()
    cb = nc.dram_tensor("cb", [2, 64], BF16, kind="ExternalInput").ap()
    out = nc.dram_tensor("out", [S, NH * 128], F32, kind="ExternalOutput").ap()

    Exp = mybir.ActivationFunctionType.Exp

    with tile.TileContext(nc) as tc, ExitStack() as ctx:
        const = ctx.enter_context(tc.tile_pool(name="const", bufs=1))
        perhead = ctx.enter_context(tc.tile_pool(name="perhead", bufs=4))
        eTp = ctx.enter_context(tc.tile_pool(name="eT", bufs=6))
        osb = ctx.enter_context(tc.tile_pool(name="osb", bufs=3))
        smal = ctx.enter_context(tc.tile_pool(name="small", bufs=2))
        scp = ctx.enter_context(tc.tile_pool(name="scores", bufs=scb, space="PSUM"))
        otp = ctx.enter_context(tc.tile_pool(name="outT", bufs=owb, space="PSUM"))
        rsp = ctx.enter_context(tc.tile_pool(name="rs", bufs=1, space="PSUM"))
        drp = ctx.enter_context(tc.tile_pool(name="dram", bufs=2, space="DRAM"))

        kT_sb = const.tile([128, S], BF16, tag="kT")
        nc.sync.dma_start(kT_sb[:], kT[:])
        v_sb = const.tile([128, S], BF16, tag="v")
        nc.sync.dma_start(v_sb[:], vR[:])
        trb_sb = const.tile([128, 128], BF16, tag="trb")
        nc.sync.dma_start(trb_sb[:], trb[:])
        idb_sb = const.tile([128, 128], BF16, tag="idb")
        nc.sync.dma_start(idb_sb[:], idb[:])
        u2_sb = const.tile([2, 128], BF16, tag="u2")
        nc.sync.dma_start(u2_sb[:], u2[:])
        ones_sb = const.tile([128, 1], BF16, tag="ones")
        nc.vector.memset(ones_sb[:], 1.0)
        onef_sb = const.tile([1, 1], F32, tag="onef")
        nc.vector.memset(onef_sb[:], 1.0)
        cb_sb = const.tile([2, 64], BF16, tag="cb")
        nc.sync.dma_start(cb_sb[:], cb[:])

        loop_cm = (tc.For_i(0, loop_n, 1,
                            hint_engines=(mybir.EngineType.PE,
                                          mybir.EngineType.Activation,
                                          mybir.EngineType.DVE,
                                          mybir.EngineType.SP))
                   if loop_n > 1 else None)
        if loop_cm is not None:
            loop_cm.__enter__()
        # Software pipeline: defer each chunk's PV/RS (and each half's
        # epilogue) by one stage so the in-order PE never sits waiting for
        # the exp of the chunk it just computed.
        pending = []
        LAG = lag

        def flush_one(force=False):
            if pending and (force or len(pending) > LAG):
                pending.pop(0)()

        for h in range(NH):
            qT_sb = perhead.tile([128, S], BF16, tag="qT")
            nc.sync.dma_start(qT_sb[:], qT[h])
            kvT_sb = perhead.tile([128, 128], BF16, tag="kvT")
            nc.sync.dma_start(kvT_sb[:], kvT[h])
            vv_sb = perhead.tile([128, 128], BF16, tag="vv")
            nc.sync.dma_start(vv_sb[:], vv[h])
            tb_sb = perhead.tile([2, HALF], BF16, tag="tb")
            nc.sync.dma_start(tb_sb[:], tb[h])

            for half in (0, 1):
                half_lo = HALF * half
                half_hi = half_lo + HALF

                # ---- plan: 512-wide scores/eT chunks + tail ----
                steps = []   # (kind, kt, a, b)
                for kt in range(NKT):
                    for (a, b) in _chunks_for(kt, half):
                        steps.append(("loc", kt, a, b))
                if half == 1:
                    steps.append(("tail", -1, 1024, 1536))
                    steps.append(("tail", -1, 1536, 2048))

                # PV/RS parts per step: split chunk at the outT 512-windows
                # and at the coverage boundary (fresh vs accumulating PSUM).
                def parts_of(kind, kt, a, b):
                    cov = half_lo
                    if kind == "tail":
                        cov = half_hi
                    elif kt > 0:
                        cov = min(max(min(1088 + 128 * (kt - 1), S), half_lo),
                                  half_hi)
                    cuts = {a, b, cov}
                    for wb in range(half_lo, half_hi + 1, WIN):
                        cuts.add(wb)
                    cuts = sorted(c for c in cuts if a <= c <= b)
                    return [(lo, hi) for lo, hi in zip(cuts, cuts[1:])
                            if lo < hi]

                n_into_w = [0, 0]
                all_parts = []
                for (kind, kt, a, b) in steps:
                    ps = parts_of(kind, kt, a, b)
                    all_parts.append(ps)
                    for (lo, hi) in ps:
                        n_into_w[(lo - half_lo) // WIN] += 1

                ow = [otp.tile([128, WIN], F32, tag="ow", name=f"ow{w}")
                      for w in range(2)]
                rs_c = rsp.tile([128, WIN], F32, tag="rs", name="rs_c")
                w_started = [False, False]
                w_seen = [0, 0]

                # ---- emit (stage A now, stage B deferred) ----
                for (kind, kt, a, b), ps in zip(steps, all_parts):
                    n = b - a
                    sc = scp.tile([128, WIN], F32, tag="sc")
                    if kind == "loc":
                        lhs_qk = kT_sb[:, 128 * kt:128 * kt + 128]
                        has_tri = a == 128 * kt and kt // 8 == half
                        has_cor = kt <= 7 and b == 128 * kt + 1088
                        has_tail = False
                    else:
                        lhs_qk = kvT_sb[:]
                        has_tri = has_cor = False
                        has_tail = True
                    writers = [("qk", 0, n)]
                    if has_tri:
                        writers.append(("tri", 0, 128))
                    if has_cor:
                        rel = (128 * kt + 1024) - a
                        writers.append(("cor", rel, rel + 64))
                    if has_tail:
                        writers.append(("tb", 0, n))
                    for i, (wkind, s0, s1) in enumerate(writers):
                        sp_f = i == len(writers) - 1
                        if wkind == "qk":
                            nc.tensor.matmul(sc[:, s0:s1], lhs_qk,
                                             qT_sb[:, a + s0:a + s1],
                                             start=True, stop=sp_f)
                        elif wkind == "tri":
                            nc.tensor.matmul(sc[:, s0:s1], idb_sb[:],
                                             trb_sb[:], start=False,
                                             stop=sp_f)
                        elif wkind == "cor":
                            nc.tensor.matmul(sc[:, s0:s1], u2_sb[:],
                                             cb_sb[:], start=False,
                                             stop=sp_f)
                        else:
                            nc.tensor.matmul(
                                sc[:, s0:s1], u2_sb[:],
                                tb_sb[:, a - HALF + s0:a - HALF + s1],
                                start=False, stop=sp_f)

                    eT = eTp.tile([128, WIN], BF16, tag="eT")
                    nc.scalar.activation(eT[:, 0:n], sc[:, 0:n], Exp,
                                         scale=SCALE)
                    lhs_pv = (v_sb[:, 128 * kt:128 * kt + 128]
                              if kind == "loc" else vv_sb[:])

                    def stage_b(ps=ps, a=a, eT=eT, lhs_pv=lhs_pv, ow=ow,
                                rs_c=rs_c, w_started=w_started, w_seen=w_seen,
                                n_into_w=n_into_w, half_lo=half_lo):
                        for (lo, hi) in ps:
                            w = (lo - half_lo) // WIN
                            wl0 = half_lo + WIN * w
                            st = not w_started[w]
                            w_started[w] = True
                            w_seen[w] += 1
                            sp_f = w_seen[w] == n_into_w[w]
                            nc.tensor.matmul(ow[w][:, lo - wl0:hi - wl0],
                                             lhs_pv, eT[:, lo - a:hi - a],
                                             start=st, stop=sp_f)
                            nc.tensor.matmul(
                                rs_c[32 * w:32 * w + 1, lo - wl0:hi - wl0],
                                ones_sb[:, 0:1], eT[:, lo - a:hi - a],
                                start=st, stop=sp_f,
                                tile_position=(0, 32 * w) if w else None)

                    flush_one()
                    pending.append(stage_b)

                def epilogue(h=h, half_lo=half_lo, ow=ow, rs_c=rs_c):
                    for w in range(2):
                        rs_row = smal.tile([1, WIN], F32, tag="rsrow",
                                           name=f"rsrow{w}")
                        nc.vector.tensor_copy(rs_row[0:1, :],
                                              rs_c[32 * w:32 * w + 1, :])
                        rsT = scp.tile([128, 4], F32, tag="sc", name=f"rsT{w}")
                        for j in range(4):
                            nc.tensor.transpose(
                                rsT[:, j:j + 1],
                                rs_row[0:1, 128 * j:128 * j + 128], onef_sb[:])
                        rcp = smal.tile([128, 4], F32, tag="rcp",
                                        name=f"rcp{w}")
                        nc.vector.reciprocal(rcp[:], rsT[:])
                        ocp = osb.tile([128, WIN], BF16, tag="ocp")
                        nc.vector.tensor_copy(ocp[:], ow[w][:])
                        os_c = osb.tile([128, WIN], F32, tag="os")
                        for j in range(4):
                            tp = scp.tile([128, 128], BF16, tag="sc",
                                          name=f"tp{w}{j}")
                            nc.tensor.transpose(
                                tp[:], ocp[:, 128 * j:128 * j + 128],
                                idb_sb[:])
                            nc.vector.tensor_scalar_mul(
                                os_c[:, 128 * j:128 * j + 128], tp[:],
                                rcp[:, j:j + 1])
                        q0 = half_lo + WIN * w
                        nc.sync.dma_start(
                            out[q0:q0 + WIN, 128 * h:128 * h + 128]
                            .rearrange("(j p) d -> p j d", p=128),
                            os_c[:].rearrange("p (j d) -> p j d", j=4))

                pending.append(epilogue)
        while pending:
            flush_one(force=True)
        if loop_cm is not None:
            loop_cm.__exit__(None, None, None)
    nc.compile()
    return nc


def make_core_inputs(query, key, value, core):
    """Host-side prep of one core's input map (bf16, pre-transposed/gathered)."""
    q3 = query.reshape(S, H, D)
    k3 = key.reshape(S, HKV, D)
    v3 = value.reshape(S, HKV, D)
    r = core
    K = k3[:, r, :]                     # [S, 128]
    V = v3[:, r, :]
    KT = np.ascontiguousarray(K.T)      # [128, S]
    vRe = np.ascontiguousarray(
        V.reshape(NKT, 128, D).transpose(1, 0, 2).reshape(128, S))

    NEG = np.float32(-100000.0)
    qT = np.empty((NH, 128, S), NPBF16)
    kvT = np.empty((NH, 128, 128), NPBF16)
    vv = np.empty((NH, 128, 128), NPBF16)
    tbias = np.zeros((NH, 2, HALF), NPBF16)
    for hl in range(NH):
        hg = NH * r + hl
        c = (7 - hg) % 8
        qT[hl] = q3[:, hg, :].T.astype(NPBF16)
        kvT[hl, :, 0:64] = KT[:, 64 * c:64 * c + 64].astype(NPBF16)
        kvT[hl, :, 64:128] = KT[:, 64 * (c + 8):64 * (c + 8) + 64].astype(NPBF16)
        vv[hl, 0:64, :] = V[64 * c:64 * c + 64, :].astype(NPBF16)
        vv[hl, 64:128, :] = V[64 * (c + 8):64 * (c + 8) + 64, :].astype(NPBF16)
        qq = np.arange(HALF)
        tbias[hl, 0, :] = np.where(qq < 64 * c, NEG, 0.0).astype(NPBF16)
        tbias[hl, 1, :] = np.where(qq < 512 + 64 * c, NEG, 0.0).astype(NPBF16)

    kk = np.arange(128)[:, None]
    qq = np.arange(128)[None, :]
    tribias = np.where(qq >= kk, 0.0, NEG).astype(NPBF16)
    u2 = np.zeros((2, 128), NPBF16)
    u2[0, 0:64] = 1.0
    u2[1, 64:128] = 1.0

    return {
        "qT": qT,
        "kT": KT.astype(NPBF16),
        "vR": vRe.astype(NPBF16),
        "kvT": kvT,
        "vv": vv,
        "tbias": tbias,
        "tribias": tribias,
        "identb": np.eye(128, dtype=NPBF16),
        "u2": u2,
        "cb": np.concatenate([np.full((1, 64), NEG, NPBF16),
                              np.zeros((1, 64), NPBF16)], axis=0),
    }


_PROGRAM = None


def _get_program():
    global _PROGRAM
    if _PROGRAM is None:
        _PROGRAM = build_program()
    return _PROGRAM


def run(query, key, value, trace=False):
    """Returns (output [S, H*D] f32, BassKernelResults)."""
    nc = _get_program()
    in_maps = [make_core_inputs(query, key, value, r) for r in range(NCORES)]
    br = run_bass_kernel_spmd(nc, in_maps, list(range(NCORES)), trace=trace)
    outp = np.hstack([br.results[r]["out"] for r in range(NCORES)])
    return outp, br


def kernel(query, key, value):
    outp, _ = run(np.asarray(query), np.asarray(key), np.asarray(value))
    return outp


# revision 21
# speedup vs baseline: 1.3860x; 1.0538x over previous
"""Block-sparse flash attention (Phi-3-small pattern) on 8 Trainium2 cores.

Problem: S=2048 tokens, 32 query heads, 8 KV heads (GQA x4), D=128,
sparse_block_size=64, local_blocks=16, vert_stride=8, per-head vertical
offset (homo_head=False).

Sharding: tensor-parallel over heads. Core r owns contiguous heads
[4r, 4r+4), which all share GQA KV head r.

Per-head block mask (head h, c = (7-h) % 8):
  block (qb, kb) active iff qb >= kb and (qb-kb < 16 or kb % 8 == c)
Decomposition used here (verified exact vs reference on host):
  - LOCAL pass, k-tile kt covers kbs {2kt, 2kt+1}: q in [128kt, 128kt+1088)
      * causal triangle inside the diagonal 128x128 block
      * -inf on k-rows [0:64) for the last 64 q cols (qb-kb == 16 corner)
  - TAIL pass: the two vertical kbs {c, c+8} gathered on the host into one
    128-row k-tile; q in [1024, 2048) with a per-head rank-2 additive bias
    (rows 0:64 active for q >= 1024+64c, rows 64:128 for q >= 1536+64c).

All masks are applied as additive -1e5 biases ACCUMULATED INTO THE SCORES
PSUM BY PE MATMULS (identity x tribias for the triangle; rank-1/2 biases
for corner/tail), which keeps every instruction within the hardware's
sync-wait slot budget.

Kernel math (scores bounded, so softmax without max-subtraction is exact
to ~1e-6):  scoresT[k,q] on PE (contraction D=128 on partitions, so PV
needs no transposes), E^T = exp(SCALE*scoresT) on ACT (bf16, 1024-wide
chunks to amortize ACT instruction overhead), out^T accumulated in PSUM
over k-tiles, rowsum via ones-matmul, final PE transpose (bf16) +
per-partition 1/rowsum scale on DVE.

All per-head pattern differences are input DATA, so the single SPMD
program is identical on all 8 cores.
"""

import sys
from contextlib import ExitStack

import numpy as np

for _p in ("/opt/trn_rl_repo", "/root/.axon_site/_ro/trn_rl_repo"):
    if _p not in sys.path:
        sys.path.append(_p)

import ml_dtypes

import concourse.bass as bass
import concourse.bacc as bacc
import concourse.mybir as mybir
import concourse.tile as tile
from concourse.bass_utils import run_bass_kernel_spmd

S = 2048
D = 128
H = 32
HKV = 8
NCORES = 8
NH = H // NCORES          # heads per core = 4
SCALE = 0.08838834764831845
NKT = S // 128            # 16 k-tiles of 128 rows
SPAN = 1088               # local window cols per k-tile (17 blocks of 64)
HALF = 1024
WIN = 512                 # PSUM bank window (outT / rs)

BF16 = mybir.dt.bfloat16
F32 = mybir.dt.float32
NPBF16 = ml_dtypes.bfloat16


def _chunks_for(kt, half):
    """512-aligned chunks of the local span of k-tile kt inside a q-half."""
    lo = max(128 * kt, HALF * half)
    hi = min(128 * kt + SPAN, S, HALF * half + HALF)
    res = []
    a = lo
    while a < hi:
        b = min((a // WIN + 1) * WIN, hi)
        res.append((a, b))
        a = b
    return res


def build_program(loop_n=1, lag=2, scb=4, owb=3, wide=False, eTb=6, osbb=3):
    nc = bacc.Bacc("TRN2", target_bir_lowering=False, debug=False)
    qT = nc.dram_tensor("qT", [NH, 128, S], BF16, kind="ExternalInput").ap()
    kT = nc.dram_tensor("kT", [128, S], BF16, kind="ExternalInput").ap()
    vR = nc.dram_tensor("vR", [128, S], BF16, kind="ExternalInput").ap()
    kvT = nc.dram_tensor("kvT", [NH, 128, 128], BF16, kind="ExternalInput").ap()
    vv = nc.dram_tensor("vv", [NH, 128, 128], BF16, kind="ExternalInput").ap()
    tm = nc.dram_tensor("tmask", [NH, 128, HALF], BF16, kind="ExternalInput").ap()
    tri = nc.dram_tensor("tri", [128, 128], BF16, kind="ExternalInput").ap()
    idb = nc.dram_tensor("identb", [128, 128], BF16, kind="ExternalInput").ap()
    out = nc.dram_tensor("out", [S, NH * 128], F32, kind="ExternalOutput").ap()

    Exp = mybir.ActivationFunctionType.Exp

    with tile.TileContext(nc) as tc, ExitStack() as ctx:
        const = ctx.enter_context(tc.tile_pool(name="const", bufs=1))
        perhead = ctx.enter_context(tc.tile_pool(name="perhead", bufs=4))
        eTp = ctx.enter_context(tc.tile_pool(name="eT", bufs=eTb))
        osb = ctx.enter_context(tc.tile_pool(name="osb", bufs=osbb))
        smal = ctx.enter_context(tc.tile_pool(name="small", bufs=2))
        scp = ctx.enter_context(tc.tile_pool(name="scores", bufs=scb, space="PSUM"))
        otp = ctx.enter_context(tc.tile_pool(name="outT", bufs=owb, space="PSUM"))
        rsp = ctx.enter_context(tc.tile_pool(name="rs", bufs=1, space="PSUM"))
        drp = ctx.enter_context(tc.tile_pool(name="dram", bufs=2, space="DRAM"))

        kT_sb = const.tile([128, S], BF16, tag="kT")
        nc.sync.dma_start(kT_sb[:], kT[:])
        v_sb = const.tile([128, S], BF16, tag="v")
        nc.sync.dma_start(v_sb[:], vR[:])
        tri_sb = const.tile([128, 128], BF16, tag="tri")
        nc.sync.dma_start(tri_sb[:], tri[:])
        idb_sb = const.tile([128, 128], BF16, tag="idb")
        nc.sync.dma_start(idb_sb[:], idb[:])
        ones_sb = const.tile([128, 1], BF16, tag="ones")
        nc.vector.memset(ones_sb[:], 1.0)
        onef_sb = const.tile([1, 1], F32, tag="onef")
        nc.vector.memset(onef_sb[:], 1.0)


        loop_cm = (tc.For_i(0, loop_n, 1,
                            hint_engines=(mybir.EngineType.PE,
                                          mybir.EngineType.Activation,
                                          mybir.EngineType.DVE,
                                          mybir.EngineType.SP))
                   if loop_n > 1 else None)
        if loop_cm is not None:
            loop_cm.__enter__()
        # Software pipeline: defer each chunk's PV/RS (and each half's
        # epilogue) by one stage so the in-order PE never sits waiting for
        # the exp of the chunk it just computed.
        pending = []
        LAG = lag

        def flush_one(force=False):
            if pending and (force or len(pending) > LAG):
                pending.pop(0)()

        for h in range(NH):
            qT_sb = perhead.tile([128, S], BF16, tag="qT")
            nc.sync.dma_start(qT_sb[:], qT[h])
            kvT_sb = perhead.tile([128, 128], BF16, tag="kvT")
            nc.sync.dma_start(kvT_sb[:], kvT[h])
            vv_sb = perhead.tile([128, 128], BF16, tag="vv")
            nc.sync.dma_start(vv_sb[:], vv[h])
            tm_sb = perhead.tile([128, HALF], BF16, tag="tm")
            nc.sync.dma_start(tm_sb[:], tm[h])

            for half in (0, 1):
                half_lo = HALF * half
                half_hi = half_lo + HALF

                # ---- plan: scores/eT chunks + tail ----
                steps = []   # (kind, kt, a, b)
                for kt in range(NKT):
                    cs = _chunks_for(kt, half)
                    if wide and cs:
                        cs = [(cs[0][0], cs[-1][1])]
                    for (a, b) in cs:
                        steps.append(("loc", kt, a, b))
                if half == 1:
                    if wide:
                        steps.append(("tail", -1, 1024, 2048))
                    else:
                        steps.append(("tail", -1, 1024, 1536))
                        steps.append(("tail", -1, 1536, 2048))

                # PV/RS parts per step: split chunk at the outT 512-windows
                # and at the coverage boundary (fresh vs accumulating PSUM).
                def parts_of(kind, kt, a, b):
                    cov = half_lo
                    if kind == "tail":
                        cov = half_hi
                    elif kt > 0:
                        cov = min(max(min(1088 + 128 * (kt - 1), S), half_lo),
                                  half_hi)
                    cuts = {a, b, cov}
                    for wb in range(half_lo, half_hi + 1, WIN):
                        cuts.add(wb)
                    cuts = sorted(c for c in cuts if a <= c <= b)
                    return [(lo, hi) for lo, hi in zip(cuts, cuts[1:])
                            if lo < hi]

                n_into_w = [0, 0]
                all_parts = []
                for (kind, kt, a, b) in steps:
                    ps = parts_of(kind, kt, a, b)
                    all_parts.append(ps)
                    for (lo, hi) in ps:
                        n_into_w[(lo - half_lo) // WIN] += 1

                ow = [otp.tile([128, WIN], F32, tag="ow", name=f"ow{w}")
                      for w in range(2)]
                rs_c = rsp.tile([128, WIN], F32, tag="rs", name="rs_c")
                w_started = [False, False]
                w_seen = [0, 0]

                # ---- emit (stage A now, stage B deferred) ----
                for (kind, kt, a, b), ps in zip(steps, all_parts):
                    n = b - a
                    sc = scp.tile([128, HALF if wide else WIN], F32,
                                  tag="sc")
                    if kind == "loc":
                        lhs_qk = kT_sb[:, 128 * kt:128 * kt + 128]
                        has_tri = a == 128 * kt and kt // 8 == half
                        has_cor = kt <= 7 and b == 128 * kt + 1088
                        has_tail = False
                    else:
                        lhs_qk = kvT_sb[:]
                        has_tri = has_cor = False
                        has_tail = True
                    for s0 in range(0, n, WIN):
                        s1 = min(s0 + WIN, n)
                        nc.tensor.matmul(sc[:, s0:s1], lhs_qk,
                                         qT_sb[:, a + s0:a + s1],
                                         start=True, stop=True)
                    eT = eTp.tile([128, HALF if wide else WIN], BF16,
                                  tag="eT")
                    nc.scalar.activation(eT[:, 0:n], sc[:, 0:n], Exp,
                                         scale=SCALE)
                    if has_tri:
                        nc.vector.tensor_mul(eT[:, 0:128], eT[:, 0:128],
                                             tri_sb[:])
                    if has_cor:
                        rel = (128 * kt + 1024) - a
                        nc.vector.memset(eT[0:64, rel:rel + 64], 0.0)
                    if has_tail:
                        nc.vector.tensor_mul(eT[:, 0:n], eT[:, 0:n],
                                             tm_sb[:, a - HALF:b - HALF])
                    lhs_pv = (v_sb[:, 128 * kt:128 * kt + 128]
                              if kind == "loc" else vv_sb[:])

                    def stage_b(ps=ps, a=a, eT=eT, lhs_pv=lhs_pv, ow=ow,
                                rs_c=rs_c, w_started=w_started, w_seen=w_seen,
                                n_into_w=n_into_w, half_lo=half_lo):
                        for (lo, hi) in ps:
                            w = (lo - half_lo) // WIN
                            wl0 = half_lo + WIN * w
                            st = not w_started[w]
                            w_started[w] = True
                            w_seen[w] += 1
                            sp_f = w_seen[w] == n_into_w[w]
                            nc.tensor.matmul(ow[w][:, lo - wl0:hi - wl0],
                                             lhs_pv, eT[:, lo - a:hi - a],
                                             start=st, stop=sp_f)
                            nc.tensor.matmul(
                                rs_c[32 * w:32 * w + 1, lo - wl0:hi - wl0],
                                ones_sb[:, 0:1], eT[:, lo - a:hi - a],
                                start=st, stop=sp_f,
                                tile_position=(0, 32 * w) if w else None)

                    flush_one()
                    pending.append(stage_b)

                def epilogue(h=h, half_lo=half_lo, ow=ow, rs_c=rs_c):
                    for w in range(2):
                        rs_row = smal.tile([1, WIN], F32, tag="rsrow",
                                           name=f"rsrow{w}")
                        nc.vector.tensor_copy(rs_row[0:1, :],
                                              rs_c[32 * w:32 * w + 1, :])
                        rsT = scp.tile([128, 4], F32, tag="sc", name=f"rsT{w}")
                        for j in range(4):
                            nc.tensor.transpose(
                                rsT[:, j:j + 1],
                                rs_row[0:1, 128 * j:128 * j + 128], onef_sb[:])
                        rcp = smal.tile([128, 4], F32, tag="rcp",
                                        name=f"rcp{w}")
                        nc.vector.reciprocal(rcp[:], rsT[:])
                        ocp = osb.tile([128, WIN], BF16, tag="ocp")
                        nc.vector.tensor_copy(ocp[:], ow[w][:])
                        os_c = osb.tile([128, WIN], F32, tag="os")
                        for j in range(4):
                            tp = scp.tile([128, 128], BF16, tag="sc",
                                          name=f"tp{w}{j}")
                            nc.tensor.transpose(
                                tp[:], ocp[:, 128 * j:128 * j + 128],
                                idb_sb[:])
                            nc.vector.tensor_scalar_mul(
                                os_c[:, 128 * j:128 * j + 128], tp[:],
                                rcp[:, j:j + 1])
                        q0 = half_lo + WIN * w
                        nc.sync.dma_start(
                            out[q0:q0 + WIN, 128 * h:128 * h + 128]
                            .rearrange("(j p) d -> p j d", p=128),
                            os_c[:].rearrange("p (j d) -> p j d", j=4))

                pending.append(epilogue)
        while pending:
            flush_one(force=True)
        if loop_cm is not None:
            loop_cm.__exit__(None, None, None)
    nc.compile()
    return nc


def make_core_inputs(query, key, value, core):
    """Host-side prep of one core's input map (bf16, pre-transposed/gathered)."""
    q3 = query.reshape(S, H, D)
    k3 = key.reshape(S, HKV, D)
    v3 = value.reshape(S, HKV, D)
    r = core
    K = k3[:, r, :]                     # [S, 128]
    V = v3[:, r, :]
    KT = np.ascontiguousarray(K.T)      # [128, S]
    vRe = np.ascontiguousarray(
        V.reshape(NKT, 128, D).transpose(1, 0, 2).reshape(128, S))

    NEG = np.float32(-100000.0)
    qT = np.empty((NH, 128, S), NPBF16)
    kvT = np.empty((NH, 128, 128), NPBF16)
    vv = np.empty((NH, 128, 128), NPBF16)
    tmask = np.zeros((NH, 128, HALF), NPBF16)
    for hl in range(NH):
        hg = NH * r + hl
        c = (7 - hg) % 8
        qT[hl] = q3[:, hg, :].T.astype(NPBF16)
        kvT[hl, :, 0:64] = KT[:, 64 * c:64 * c + 64].astype(NPBF16)
        kvT[hl, :, 64:128] = KT[:, 64 * (c + 8):64 * (c + 8) + 64].astype(NPBF16)
        vv[hl, 0:64, :] = V[64 * c:64 * c + 64, :].astype(NPBF16)
        vv[hl, 64:128, :] = V[64 * (c + 8):64 * (c + 8) + 64, :].astype(NPBF16)
        qq = np.arange(HALF)
        tmask[hl, 0:64, :] = (qq >= 64 * c).astype(NPBF16)[None, :]
        tmask[hl, 64:128, :] = (qq >= 512 + 64 * c).astype(NPBF16)[None, :]

    kk = np.arange(128)[:, None]
    qq2 = np.arange(128)[None, :]
    tri = (qq2 >= kk).astype(NPBF16)

    return {
        "qT": qT,
        "kT": KT.astype(NPBF16),
        "vR": vRe.astype(NPBF16),
        "kvT": kvT,
        "vv": vv,
        "tmask": tmask,
        "tri": tri,
        "identb": np.eye(128, dtype=NPBF16),
    }


_PROGRAM = None


def _get_program():
    global _PROGRAM
    if _PROGRAM is None:
        _PROGRAM = build_program()
    return _PROGRAM


def run(query, key, value, trace=False):
    """Returns (output [S, H*D] f32, BassKernelResults)."""
    nc = _get_program()
    in_maps = [make_core_inputs(query, key, value, r) for r in range(NCORES)]
    br = run_bass_kernel_spmd(nc, in_maps, list(range(NCORES)), trace=trace)
    outp = np.hstack([br.results[r]["out"] for r in range(NCORES)])
    return outp, br


def kernel(query, key, value):
    outp, _ = run(np.asarray(query), np.asarray(key), np.asarray(value))
    return outp
